# revision 3
# baseline (speedup 1.0000x reference)
"""Trainium2 Bass kernel v6 for nn_AttentionSubModule (B=262144, Q=25, D=9).

v7 = v6 + contiguous-DMA batch mapping ((s e c) view: one 3600B descriptor
per partition per super-chunk instead of four strided 900B segments; pure
batch permutation that the y-view inverts exactly).
v6 over v3: 3-deep software-pipelined stages (scores | attnV | LN tail run
on different chunks each iteration, so DVE never stalls on Pool/ACT within
a chunk), d=9 unpadded score tree, and DVE/Pool rebalance (t2/u2/y3 moves).
Per-chunk engine busy (cost model): DVE 11.96us, Pool 8.77us, ACT 2.48us;
TimelineSim 14.1us/chunk -> ~3.6ms/core device time.


Batch-major DVE pipeline, instruction-count frugal (~35 instrs / 128 elems):
  - x [128 x 225] fp32 batch-major (super-batched DMA), cast fp16 (Pool),
    PE-transposed to x^T (ones row folded for bias).
  - ONE weights-moving projection pair (stationary = x^T halves, moving =
    static WKV [226 x 510]) -> K batch-major k-major (cols q*10+d) AND
    V batch-major d-major (cols 250 + d*26 + k, d=9 row = ones so the
    attn@V matmul also produces row sums).
  - scores: prod = k3 x k3 broadcast mult (fp16, DVE 2x) + tree-add
    reduction over d (2x where packed).  Softmax with NO max subtraction:
    diag(scores) >= 0 guarantees rowsum >= 1, bf16 E covers exp range.
  - attnV: prod2[q,d,k] = E[q,k] * Vt[d,k] (2x) + tree-add over k ->
    res[q,d] bf16 with rowsum in d=9.
  - normalize, residual with fp32 x, LayerNorm (DVE reduces, Pool
    elementwise, ACT square/sqrt/exp).
"""

import os

import numpy as np

import bass_rust as br
import concourse.bass as bass
import concourse.mybir as mybir
import concourse.tile as tile
from concourse.bass_utils import run_bass_kernel_spmd
from concourse.vector_clock import ScopedClock

B, Q, D = 262144, 25, 9
SEGS = [(0, 3), (3, 13), (13, 23), (23, 25)]
EPS = 1e-5
N_CORES = 8
CH = 128                   # elements per chunk (batch on partitions)
SUP = 4                    # chunks per super-chunk (DMA batching)
SE = SUP * CH
DP = 10                    # padded d (K cols per q; V d rows incl ones)
QP = 26                    # padded k for V d-major cols
KW = Q * DP                # 250
VW = DP * QP               # 260
PW = KW + VW               # 510

F32 = mybir.dt.float32
F16 = mybir.dt.float16
BF16 = mybir.dt.bfloat16
AX = mybir.AxisListType
OP = mybir.AluOpType
ACTF = mybir.ActivationFunctionType

def _split_multi_waits(nc, max_waits=1):
    """walrus rejects instructions with more than one sync-wait command.
    Hoist extra waits onto same-engine NOPs inserted just before the
    offending instruction (same-engine program order makes this equivalent)."""
    for bb in nc.main_func.blocks:
        insts = bb.instructions
        out = []
        changed = False
        for inst in insts:
            si = getattr(inst, "sync_info", None)
            if si is not None and len(si.on_wait) > max_waits:
                waits = list(si.on_wait)
                keep = waits[:max_waits]
                extra = waits[max_waits:]
                for w in extra:
                    nop = mybir.InstNoOp(name=f"wsplit_{nc.next_id()}", ins=[], outs=[])
                    nop.engine = inst.engine
                    nop.sync_info = br.SyncInfo(on_wait=[w], on_update=[])
                    out.append(nop)
                inst.sync_info = br.SyncInfo(on_wait=keep, on_update=list(si.on_update))
                changed = True
            out.append(inst)
        if changed:
            bb.instructions = out


def _patch_tile_drain():
    """walrus rejects >1 sync-wait on the Tile tail Drain; spread the waits
    over single-wait NOPs instead."""

    def _drain_and_barrier(self, tick_clock, wait_clock):
        nc = self.nc
        probe = nc.sync.nop(nofuse=True)
        wait_clock.add_sem_waits(probe.ins, ScopedClock({None: tick_clock.global_clock}))
        si = probe.ins.sync_info
        if si is not None and len(si.on_wait) > 1:
            waits = list(si.on_wait)
            probe.ins.sync_info = br.SyncInfo(on_wait=[waits[0]], on_update=list(si.on_update))
            for w in waits[1:]:
                n = nc.sync.nop(nofuse=True)
                n.ins.sync_info = br.SyncInfo(on_wait=[w], on_update=[])
        nc.sync.drain()

        nc.all_engine_barrier()
        assert self.sems is not None
        popped = nc._tile_sem_poison_stack.pop()
        assert popped is self._sem_poison
        nc.clear_and_free_semaphores(list(self.sems.allocated().values()))
        nc.all_engine_barrier()

    tile.TileContext._drain_and_barrier = _drain_and_barrier


_patch_tile_drain()


def _seg_of(q):
    for si, (s, e) in enumerate(SEGS):
        if s <= q < e:
            return si
    raise ValueError(q)


def make_wkv(inp):
    """Static moving matrix WKV [226 x 510] (bf16).

    Contraction rows: r = qt*9+dp for qt<25 (x features), r=225 (ones).
    Cols 0..249:    K batch-major, col q*10+d (d<9):
                    WKV[qt*9+dp, q*10+d] = Wk_seg(q)[d, dp] * (qt==q)
                    WKV[225,     q*10+d] = bk_seg(q)[d]
    Cols 250..509:  V d-major, col 250 + d*26 + k (k<25, d<9):
                    WKV[qt*9+dp, .] = Wv_seg(k)[d, dp] * (qt==k)
                    WKV[225,     .] = bv_seg(k)[d]
                    d=9 row of V = ones (rowsum fold): WKV[225, 250+9*26+k] = 1
    """
    Wk = [np.asarray(inp[n], np.float64) for n in ("W_jk", "W_ok", "W_gk", "W_bk")]
    bk = [np.asarray(inp[n], np.float64) for n in ("b_jk", "b_ok", "b_gk", "b_bk")]
    Wv = [np.asarray(inp[n], np.float64) for n in ("W_jv", "W_ov", "W_gv", "W_bv")]
    bv = [np.asarray(inp[n], np.float64) for n in ("b_jv", "b_ov", "b_gv", "b_bv")]

    WKV = np.zeros((226, PW), np.float32)
    for q in range(Q):
        s = _seg_of(q)
        for d in range(D):
            for dp in range(D):
                WKV[q * D + dp, q * DP + d] = Wk[s][d, dp]
                WKV[q * D + dp, KW + d * QP + q] = Wv[s][d, dp]
            WKV[225, q * DP + d] = bk[s][d]
            WKV[225, KW + d * QP + q] = bv[s][d]
        WKV[225, KW + D * QP + q] = 1.0  # V ones-column (rowsum)
    return WKV


def build_nc(n_el, ln_affine=True, passes=1):
    assert n_el % SE == 0
    n_ch = n_el // CH
    nc = bass.Bass("TRN2", target_bir_lowering=False, debug=False)

    x_d = nc.dram_tensor("x", [n_el, Q * D], F32, kind="ExternalInput")
    y_d = nc.dram_tensor("y", [n_el, Q * D], F32, kind="ExternalOutput")
    wkv_d = nc.dram_tensor("wkv", [226, PW], F16, kind="ExternalInput")
    id_d = nc.dram_tensor("ident", [128, 128], F16, kind="ExternalInput")
    g_d = nc.dram_tensor("ln_g", [D], F32, kind="ExternalInput")
    b_d = nc.dram_tensor("ln_b", [D], F32, kind="ExternalInput")

    # batch -> (super, partition, chunk) with chunk INNER: each partition
    # holds SUP consecutive rows, so the DMA is one contiguous 3600B
    # descriptor per partition instead of SUP strided 900B segments.
    x_v = x_d.ap().rearrange("(s e c) f -> s e c f", c=SUP, e=CH)
    y_v = y_d.ap().rearrange("(s e c) f -> s e c f", c=SUP, e=CH)

    with tile.TileContext(nc) as tc:
        with (
            tc.tile_pool(name="singles", bufs=1) as singles,
            tc.tile_pool(name="xin", bufs=3) as xin,
            tc.tile_pool(name="kv", bufs=3) as kv,
            tc.tile_pool(name="big", bufs=3) as big,
            tc.tile_pool(name="sm", bufs=4) as sm,
            tc.tile_pool(name="yio", bufs=3) as yio,
            tc.tile_pool(name="psp", bufs=2, space="PSUM") as psp,
            tc.tile_pool(name="psx", bufs=2, space="PSUM") as psx,
        ):
            wkva = singles.tile([128, PW], F16, tag="wkva")
            wkvb = singles.tile([98, PW], F16, tag="wkvb")
            nc.sync.dma_start(out=wkva, in_=wkv_d[0:128, :])
            nc.sync.dma_start(out=wkvb, in_=wkv_d[128:226, :])
            ident = singles.tile([128, 128], F16, tag="ident")
            nc.sync.dma_start(out=ident, in_=id_d[:, :])
            g_rep = singles.tile([128, D], F32, tag="g_rep")
            b_rep = singles.tile([128, D], F32, tag="b_rep")
            nc.gpsimd.dma_start(out=g_rep, in_=g_d.ap().partition_broadcast(128))
            nc.gpsimd.dma_start(out=b_rep, in_=b_d.ap().partition_broadcast(128))
            eps_t = singles.tile([128, 1], F32, tag="eps")
            nc.vector.memset(eps_t, EPS)
            xta = [
                singles.tile([128, CH], F16, tag=f"xta{j}", name=f"xta{j}")
                for j in range(2)
            ]
            xtb = [
                singles.tile([98, CH], F16, tag=f"xtb{j}", name=f"xtb{j}")
                for j in range(2)
            ]
            for j in range(2):
                nc.gpsimd.memset(xtb[j], 1.0)

            sup_tiles = {}
            st = {}
            st2 = {}
            st3 = {}

            def stage_a(c):
                s, c4 = c // SUP, c % SUP
                buf = c % 2
                if c4 == 0:
                    xfs = xin.tile([CH, SUP, Q * D], F32, tag="xfs", name="xfs")
                    nc.sync.dma_start(out=xfs, in_=x_v[s])
                    y_sup = yio.tile(
                        [CH, SUP, Q * D], F32, tag="y_sup", name="y_sup"
                    )
                    sup_tiles[s] = (xfs, y_sup)
                xfs, y_sup = sup_tiles[s]
                xf = xfs[:, c4, :]

                xb = xin.tile([CH, Q * D], F16, tag="xb", name="xb")
                nc.gpsimd.tensor_copy(out=xb, in_=xf)
                px = psx.tile([128, 2 * CH], F16, tag="px", name="px")
                nc.tensor.transpose(px[0:128, 0:CH], xb[:, 0:128], ident)
                nc.tensor.transpose(px[0:97, CH : 2 * CH], xb[:, 128:225], ident)
                nc.scalar.copy(out=xta[buf], in_=px[0:128, 0:CH])
                nc.scalar.copy(out=xtb[buf][0:97, :], in_=px[0:97, CH : 2 * CH])

                pkv = psp.tile([CH, PW], F32, tag="pkv", name="pkv")
                nc.tensor.matmul(pkv, xta[buf], wkva, start=True, stop=False)
                nc.tensor.matmul(pkv, xtb[buf], wkvb, start=False, stop=True)
                kvbm = kv.tile([CH, PW], F16, tag="kvbm", name="kvbm")
                nc.scalar.copy(out=kvbm, in_=pkv)
                st[c] = dict(
                    kvbm=kvbm,
                    xf3=xf.rearrange("e (q d) -> e q d", d=D),
                    y3=y_sup[:, c4, :].rearrange("e (q d) -> e q d", d=D),
                )

            def stage_b1(c):
                d_ = st.pop(c)
                kvbm = d_["kvbm"]

                # scores: prod + tree reduce over d=9 (fp16, no pad read)
                k39 = kvbm[:, 0:KW].rearrange("e (q d) -> e q d", d=DP)[:, :, 0:9]
                prod = big.tile([128, Q, Q, 9], F16, tag="prod", name="prod")
                nc.vector.tensor_tensor(
                    out=prod,
                    in0=k39.unsqueeze(2).broadcast_to((128, Q, Q, 9)),
                    in1=k39.unsqueeze(1).broadcast_to((128, Q, Q, 9)),
                    op=OP.mult,
                )
                t1 = big.tile([128, Q, Q, 4], F16, tag="t1", name="t1")
                nc.vector.tensor_tensor(
                    out=t1, in0=prod[:, :, :, 0:4], in1=prod[:, :, :, 4:8], op=OP.add
                )
                t2 = big.tile([128, Q, Q, 2], F16, tag="t2", name="t2")
                nc.gpsimd.tensor_tensor(
                    out=t2, in0=t1[:, :, :, 0:2], in1=t1[:, :, :, 2:4], op=OP.add
                )
                sc = big.tile([128, Q, Q], F16, tag="sc", name="sc")
                nc.gpsimd.tensor_tensor(
                    out=sc, in0=t2[:, :, :, 0], in1=t2[:, :, :, 1], op=OP.add
                )
                nc.gpsimd.tensor_tensor(out=sc, in0=sc, in1=prod[:, :, :, 8], op=OP.add)

                # E = exp(scores) in bf16 (no max subtraction needed)
                e_t = big.tile([128, Q, Q], BF16, tag="e_t", name="e_t")
                nc.scalar.activation(out=e_t, in_=sc, func=ACTF.Exp)
                d_["e_t"] = e_t
                st2[c] = d_

            def stage_b2(c):
                d_ = st2.pop(c)
                e_t, xf3, y3 = d_["e_t"], d_["xf3"], d_["y3"]
                kvbm = d_["kvbm"]
                vt = kvbm[:, KW:PW].rearrange("e (d k) -> e d k", k=QP)

                # attnV: prod2[q, d, k] = E[q,k] * Vt[d,k]; tree over k
                prod2 = big.tile([128, Q, DP, Q], BF16, tag="prod2", name="prod2")
                nc.vector.tensor_tensor(
                    out=prod2,
                    in0=e_t.unsqueeze(2).broadcast_to((128, Q, DP, Q)),
                    in1=vt[:, :, 0:Q].unsqueeze(1).broadcast_to((128, Q, DP, Q)),
                    op=OP.mult,
                )
                u1 = big.tile([128, Q, DP, 12], BF16, tag="u1", name="u1")
                nc.vector.tensor_tensor(
                    out=u1,
                    in0=prod2[:, :, :, 0:12],
                    in1=prod2[:, :, :, 12:24],
                    op=OP.add,
                )
                u2 = big.tile([128, Q, DP, 6], BF16, tag="u2", name="u2")
                nc.vector.tensor_tensor(
                    out=u2, in0=u1[:, :, :, 0:6], in1=u1[:, :, :, 6:12], op=OP.add
                )
                u3 = big.tile([128, Q, DP, 3], BF16, tag="u3", name="u3")
                nc.vector.tensor_tensor(
                    out=u3, in0=u2[:, :, :, 0:3], in1=u2[:, :, :, 3:6], op=OP.add
                )
                res = sm.tile([128, Q, DP], F32, tag="res", name="res")
                nc.vector.tensor_tensor(
                    out=res, in0=u3[:, :, :, 0], in1=u3[:, :, :, 1], op=OP.add
                )
                nc.gpsimd.tensor_tensor(out=res, in0=res, in1=u3[:, :, :, 2], op=OP.add)
                nc.gpsimd.tensor_tensor(
                    out=res, in0=res, in1=prod2[:, :, :, 24], op=OP.add
                )

                rin = sm.tile([128, Q], F32, tag="rin", name="rin")
                nc.vector.reciprocal(out=rin, in_=res[:, :, D])
                resn = sm.tile([128, Q, D], F32, tag="resn", name="resn")
                nc.vector.tensor_tensor(
                    out=resn,
                    in0=res[:, :, 0:D],
                    in1=rin.unsqueeze(2).broadcast_to((128, Q, D)),
                    op=OP.mult,
                )
                d_["resn"] = resn
                st3[c] = d_

            def stage_b3(c):
                d_ = st3.pop(c)
                resn, xf3, y3 = d_["resn"], d_["xf3"], d_["y3"]
                yt = sm.tile([128, Q, D], F32, tag="yt", name="yt")
                nc.gpsimd.tensor_tensor(out=yt, in0=resn, in1=xf3, op=OP.add)

                msum = sm.tile([128, Q], F32, tag="msum", name="msum")
                nc.vector.tensor_reduce(out=msum, in_=yt, axis=AX.X, op=OP.add)
                negmean = sm.tile([128, Q], F32, tag="negmean", name="negmean")
                nc.gpsimd.tensor_scalar_mul(negmean, msum, -1.0 / D)
                yc = sm.tile([128, Q, D], F32, tag="yc", name="yc")
                nc.gpsimd.tensor_tensor(
                    out=yc,
                    in0=yt,
                    in1=negmean.unsqueeze(2).broadcast_to((128, Q, D)),
                    op=OP.add,
                )
                sq = sm.tile([128, Q, D], F32, tag="sq", name="sq")
                nc.scalar.activation(out=sq, in_=yc, func=ACTF.Square)
                vsum = sm.tile([128, Q], F32, tag="vsum", name="vsum")
                nc.vector.tensor_reduce(out=vsum, in_=sq, axis=AX.X, op=OP.add)
                sd = sm.tile([128, Q], F32, tag="sd", name="sd")
                nc.scalar.activation(
                    out=sd, in_=vsum, func=ACTF.Sqrt, bias=eps_t, scale=1.0 / D
                )
                sdinv = sm.tile([128, Q], F32, tag="sdinv", name="sdinv")
                nc.vector.reciprocal(out=sdinv, in_=sd)
                if not ln_affine:
                    nc.gpsimd.tensor_tensor(
                        out=y3,
                        in0=yc,
                        in1=sdinv.unsqueeze(2).broadcast_to((128, Q, D)),
                        op=OP.mult,
                    )
                else:
                    tA = sm.tile([128, Q, D], F32, tag="tA", name="tA")
                    nc.vector.tensor_tensor(
                        out=tA,
                        in0=yc,
                        in1=sdinv.unsqueeze(2).broadcast_to((128, Q, D)),
                        op=OP.mult,
                    )
                    tB = sm.tile([128, Q, D], F32, tag="tB", name="tB")
                    nc.gpsimd.tensor_tensor(
                        out=tB,
                        in0=tA,
                        in1=g_rep.unsqueeze(1).broadcast_to((128, Q, D)),
                        op=OP.mult,
                    )
                    nc.vector.tensor_tensor(
                        out=y3,
                        in0=tB,
                        in1=b_rep.unsqueeze(1).broadcast_to((128, Q, D)),
                        op=OP.add,
                    )
                s, c4 = c // SUP, c % SUP
                if c4 == SUP - 1:
                    _, y_sup = sup_tiles[s]
                    nc.sync.dma_start(out=y_v[s], in_=y_sup)

            for _p in range(passes):
                st.clear()
                st2.clear()
                st3.clear()
                sup_tiles.clear()
                for t in range(n_ch + 3):
                    if t < n_ch:
                        stage_a(t)
                    if 0 <= t - 1 < n_ch:
                        stage_b1(t - 1)
                    if 0 <= t - 2 < n_ch:
                        stage_b2(t - 2)
                    if 0 <= t - 3 < n_ch:
                        stage_b3(t - 3)

    _split_multi_waits(nc)
    return nc


def _host_inputs(inputs):
    x = np.ascontiguousarray(np.asarray(inputs["x"], np.float32).reshape(-1, Q * D))
    wkv = make_wkv(inputs).astype(np.float16)
    ident = np.eye(128, dtype=np.float16)
    g = np.asarray(inputs["ln_g"], np.float32)
    b = np.asarray(inputs["ln_b"], np.float32)
    return x, wkv, ident, g, b


def kernel(**inputs):
    x, wkv, ident, g, b = _host_inputs(inputs)
    n_el_total = x.shape[0]
    assert n_el_total % (N_CORES * SE) == 0
    bc = n_el_total // N_CORES

    ln_affine = not (np.all(g == 1.0) and np.all(b == 0.0))
    nc = build_nc(bc, ln_affine=ln_affine)
    in_maps = []
    for i in range(N_CORES):
        in_maps.append(
            {
                "x": x[i * bc : (i + 1) * bc],
                "wkv": wkv,
                "ident": ident,
                "ln_g": g,
                "ln_b": b,
            }
        )
    rr = run_bass_kernel_spmd(
        nc, in_maps, list(range(N_CORES)), tmpdir=os.environ.get("BASS_TMPDIR")
    )
    globals()["LAST_RR"] = rr
    globals()["LAST_NC"] = nc
    globals()["LAST_IN_MAPS"] = in_maps
    y = np.concatenate([rr.results[i]["y"] for i in range(N_CORES)], axis=0)
    return y.reshape(np.asarray(inputs["x"]).shape)

